# revision 21
# baseline (speedup 1.0000x reference)
"""Cross multi-head attention on 8 Trainium2 NeuronCores.

Sharding: tensor-parallel by heads within a batch. Core c handles batch
b = c//2 and heads hh*8:(hh+1)*8 with hh = c%2, full S query rows. Each
core emits a partial output projection (its heads' slice of the
combo_linear contraction); the host sums the two partials per batch
(the unshard step for TP on the combo_linear input dim). No K/V
recompute and no on-device collectives.

Per-core kernel (all in T-major layout so no on-chip transposes at all;
the host pre-transposes x/y and pre-packs the weights):
  QT[hp]  [128,Sl] = Wq2[hp].T @ xT      (head-pair packed: rows 0:64 head a,
  KT[hp]  [128,T]  = Wk2[hp].T @ yT       rows 64:128 head b; bias fused into
                                          the PSUM->SBUF copy on DVE)
  V'      [t,h,65] = yT.T @ Wv_cat | 1   (natural layout + ones column)
  scoresT [t,q]    = KT_h.T @ QT_h       (K=64 contraction, two heads run
                                          concurrently via PE row tiling)
  expT    = exp(scoresT * 0.125)         (one ACT op per psum pair tile)
  oT'     += V'_h.T @ expT               (M=65: row 64 accumulates the softmax
                                          denominator for free)
  oT      = oT'[0:64] * bcast(1/oT'[64]) (deferred normalization, one DVE
                                          pass; partition-shifted for head b)
  out     = concatT.T @ Wo + bo
Matmul inputs bf16 (fp32 PSUM accumulation), softmax in fp32.
"""

import numpy as np

B, S, T, E, H, D = 4, 2048, 2048, 1024, 16, 64
N_CORES = 8

_compiled = {}


def _dt():
    from concourse import mybir

    return mybir.dt


def _mybir():
    from concourse import mybir

    return mybir


def build_program(n_hp=4, s_loc=2048, t_len=2048, n_et=8, debug_taps=False):
    """Emit the per-core bass program. Sizes parameterizable for sim tests.

    n_hp: head pairs (heads = 2*n_hp), s_loc: query rows on this core,
    t_len: key rows, n_et: contraction tiles (emb dim = 128*n_et).
    """
    import concourse.tile as tile
    from concourse import bacc

    dt = _dt()
    bf16 = dt.bfloat16
    f32 = dt.float32

    e_dim = 128 * n_et
    c_dim = 128 * n_hp  # concat dim on this core's heads
    n_h = 2 * n_hp
    n_tt = t_len // 128  # key tiles
    qch = min(512, s_loc)  # query chunk width
    tch = min(512, t_len)
    ech = min(512, e_dim)
    vch = min(512, c_dim)
    n_qc = s_loc // qch  # query chunks for attention
    n_st = s_loc // 128  # output row tiles
    n_ec = e_dim // ech  # output col chunks

    nc = bacc.Bacc("TRN2", target_bir_lowering=False, debug=False)

    # ---- DRAM I/O (host provides these layouts directly) ----
    xT = nc.dram_tensor("xT", [128, n_et, s_loc], bf16, kind="ExternalInput").ap()
    yT = nc.dram_tensor("yT", [128, n_et, t_len], bf16, kind="ExternalInput").ap()
    wq2 = nc.dram_tensor("wq2", [128, n_hp, n_et, 128], bf16, kind="ExternalInput").ap()
    wk2 = nc.dram_tensor("wk2", [128, n_hp, n_et, 128], bf16, kind="ExternalInput").ap()
    wv = nc.dram_tensor("wv", [128, n_et, c_dim], bf16, kind="ExternalInput").ap()
    wo = nc.dram_tensor("wo", [128, n_hp, e_dim], bf16, kind="ExternalInput").ap()
    bqc = nc.dram_tensor("bqc", [128, n_hp], f32, kind="ExternalInput").ap()
    bkc = nc.dram_tensor("bkc", [128, n_hp], f32, kind="ExternalInput").ap()
    bvc = nc.dram_tensor("bvc", [1, c_dim], bf16, kind="ExternalInput").ap()
    bo_in = nc.dram_tensor("bo_in", [1, e_dim], f32, kind="ExternalInput").ap()
    out = nc.dram_tensor("out", [s_loc, e_dim], f32, kind="ExternalOutput").ap()

    from contextlib import ExitStack

    dbg = {}
    if debug_taps:
        dbg["v"] = nc.dram_tensor(
            "dbg_v", [128, n_tt, n_h, 65], dt.bfloat16, kind="ExternalOutput"
        ).ap()
        dbg["oT"] = nc.dram_tensor(
            "dbg_oT", [128, n_hp, s_loc], dt.bfloat16, kind="ExternalOutput"
        ).ap()
        dbg["qt0"] = nc.dram_tensor(
            "dbg_qt0", [128, s_loc], dt.bfloat16, kind="ExternalOutput"
        ).ap()
        dbg["kt0"] = nc.dram_tensor(
            "dbg_kt0", [128, t_len], dt.bfloat16, kind="ExternalOutput"
        ).ap()
        dbg["rcp0"] = nc.dram_tensor(
            "dbg_rcp0", [33, qch], dt.float32, kind="ExternalOutput"
        ).ap()
        dbg["exp0"] = nc.dram_tensor(
            "dbg_exp0", [128, 2, qch], dt.bfloat16, kind="ExternalOutput"
        ).ap()

    with tile.TileContext(nc) as tc, ExitStack() as ctx:
        consts = ctx.enter_context(tc.tile_pool(name="consts", bufs=1))
        scr_pool = ctx.enter_context(tc.tile_pool(name="scr", bufs=2, space="DRAM"))
        qt_pool = ctx.enter_context(tc.tile_pool(name="qt", bufs=2))
        kt_pool = ctx.enter_context(tc.tile_pool(name="kt", bufs=2))
        exp_pool = ctx.enter_context(tc.tile_pool(name="expp", bufs=3))
        osc_pool = ctx.enter_context(tc.tile_pool(name="osc", bufs=4))
        rbc_pool = ctx.enter_context(tc.tile_pool(name="rbc", bufs=4))
        osb_pool = ctx.enter_context(tc.tile_pool(name="osb", bufs=2))
        sc_ps = ctx.enter_context(tc.tile_pool(name="scps", bufs=2, space="PSUM"))
        acc_ps = ctx.enter_context(tc.tile_pool(name="accps", bufs=2, space="PSUM"))
        o_ps_pool = ctx.enter_context(tc.tile_pool(name="ops", bufs=2, space="PSUM"))

        # ---- resident loads, ordered so the PE can start ASAP ----
        # Q-proj needs bqc + wq + xT: those go first on their queues. yT/wk
        # (K-proj) follow, then wv/wo. Chunked so early pieces land early.
        xT_sb = consts.tile([128, n_et, s_loc], bf16)
        wq_sb = consts.tile([128, n_hp, n_et, 128], bf16)
        yT_sb = consts.tile([128, n_et, t_len], bf16)
        wk_sb = consts.tile([128, n_hp, n_et, 128], bf16)
        wv_sb = consts.tile([128, n_et, c_dim], bf16)
        wo_sb = consts.tile([128, n_hp, e_dim], bf16)
        bo_bc = consts.tile([128, e_dim], f32)
        bqc_sb = consts.tile([128, n_hp], f32)
        bkc_sb = consts.tile([128, n_hp], f32)
        bv_sb = consts.tile([1, c_dim], bf16)
        bv_bc = consts.tile([128, c_dim], bf16)

        # one queue, strict need-order, pieces <=256KB: each descriptor
        # grabs the next free ring, so the first ~16 pieces finish in ~9us
        # instead of everything finishing together at ~28us (rings are
        # per-descriptor FIFO at ~27GB/s each)
        n_sh = max(1, s_loc // qch)
        n_th = t_len // tch

        def load_x(sh):
            for et in range(0, n_et, 2):
                nc.sync.dma_start(
                    out=xT_sb[:, et : et + 2, sh * qch : (sh + 1) * qch],
                    in_=xT[:, et : et + 2, sh * qch : (sh + 1) * qch],
                )

        def load_y(th):
            for et in range(0, n_et, 2):
                nc.sync.dma_start(
                    out=yT_sb[:, et : et + 2, th * tch : (th + 1) * tch],
                    in_=yT[:, et : et + 2, th * tch : (th + 1) * tch],
                )

        nc.sync.dma_start(out=bqc_sb, in_=bqc)
        nc.sync.dma_start(out=bkc_sb, in_=bkc)
        nc.sync.dma_start(out=bv_sb, in_=bvc)
        nc.sync.dma_start(out=wq_sb[:, 0, :, :], in_=wq2[:, 0, :, :])
        load_x(0)
        nc.sync.dma_start(out=wk_sb[:, 0, :, :], in_=wk2[:, 0, :, :])
        load_y(0)
        for et in range(0, n_et, 2):
            nc.sync.dma_start(
                out=wv_sb[:, et : et + 2, :], in_=wv[:, et : et + 2, :]
            )
        load_y(1)
        load_x(1)
        for th in range(2, n_th):
            load_y(th)
        for hp in range(1, n_hp):
            nc.sync.dma_start(out=wq_sb[:, hp, :, :], in_=wq2[:, hp, :, :])
            nc.sync.dma_start(out=wk_sb[:, hp, :, :], in_=wk2[:, hp, :, :])
        for sh in range(2, n_sh):
            load_x(sh)
        for hp in range(n_hp):
            nc.sync.dma_start(out=wo_sb[:, hp, :], in_=wo[:, hp, :])
        nc.gpsimd.dma_start(out=bv_bc, in_=bvc[0:1, :].to_broadcast([128, c_dim]))
        nc.gpsimd.dma_start(out=bo_bc, in_=bo_in[0:1, :].to_broadcast([128, e_dim]))

        # V' with a ones column per head: [p, tt, head, 65]
        v_sb = consts.tile([128, n_tt, n_h, 65], bf16)
        nc.vector.memset(v_sb[:, :, :, 64:65], 1.0)
        oT_all = consts.tile([128, n_hp, s_loc], bf16)

        def v_proj_tile(vc, tt):
            # V[:, vc*vch : (vc+1)*vch] for key tile tt, natural [t, c]
            # layout; bias is folded into the PSUM->SBUF evacuation via a
            # partition-broadcast add on DVE (saves a rank-1 PE matmul)
            nhc = vch // 64  # heads covered by this chunk
            h0 = vc * nhc
            ps = acc_ps.tile([128, vch], f32, tag="acc")
            for et in range(n_et):
                nc.tensor.matmul(
                    out=ps,
                    lhsT=yT_sb[:, et, tt * 128 : (tt + 1) * 128],
                    rhs=wv_sb[:, et, vc * vch : (vc + 1) * vch],
                    start=(et == 0),
                    stop=(et == n_et - 1),
                )
            nc.vector.tensor_add(
                v_sb[:, tt, h0 : h0 + nhc, 0:64],
                ps.rearrange("p (h d) -> p h d", d=64),
                bv_bc[:, vc * vch : (vc + 1) * vch].rearrange(
                    "p (h d) -> p h d", d=64
                ),
            )

        def out_proj_block(st):
            # out rows [st*128, (st+1)*128) = concatT.T @ Wo + bo; emitted
            # per qc block inside the last hp iteration so only the final
            # block trails the attention loop
            for ec in range(n_ec):
                ps = acc_ps.tile([128, ech], f32, tag="acc")
                for ct in range(n_hp):
                    nc.tensor.matmul(
                        out=ps,
                        lhsT=oT_all[:, ct, st * 128 : (st + 1) * 128],
                        rhs=wo_sb[:, ct, ec * ech : (ec + 1) * ech],
                        start=(ct == 0),
                        stop=(ct == n_hp - 1),
                    )
                o_sb = osb_pool.tile([128, ech], f32, tag="osb")
                nc.vector.tensor_add(o_sb, ps, bo_bc[:, ec * ech : (ec + 1) * ech])
                # two half-width DMAs so the write spreads over two rings
                # (a single 256KB descriptor drains one ring for ~9us,
                # which would dominate the kernel tail)
                eh = ech // 2
                for h in range(2):
                    nc.sync.dma_start(
                        out=out[
                            st * 128 : (st + 1) * 128,
                            ec * ech + h * eh : ec * ech + (h + 1) * eh,
                        ],
                        in_=o_sb[:, h * eh : (h + 1) * eh],
                    )

        def q_chunk(hp, qt, sc):
            ps = acc_ps.tile([128, qch], f32, tag="acc")
            for et in range(n_et):
                nc.tensor.matmul(
                    out=ps,
                    lhsT=wq_sb[:, hp, et, :],
                    rhs=xT_sb[:, et, sc * qch : (sc + 1) * qch],
                    start=(et == 0),
                    stop=(et == n_et - 1),
                )
            nc.vector.tensor_scalar_add(
                out=qt[:, sc * qch : (sc + 1) * qch],
                in0=ps,
                scalar1=bqc_sb[:, hp : hp + 1],
            )

        def k_chunk(hp, kt, tc_):
            ps = acc_ps.tile([128, tch], f32, tag="acc")
            for et in range(n_et):
                nc.tensor.matmul(
                    out=ps,
                    lhsT=wk_sb[:, hp, et, :],
                    rhs=yT_sb[:, et, tc_ * tch : (tc_ + 1) * tch],
                    start=(et == 0),
                    stop=(et == n_et - 1),
                )
            nc.vector.tensor_scalar_add(
                out=kt[:, tc_ * tch : (tc_ + 1) * tch],
                in0=ps,
                scalar1=bkc_sb[:, hp : hp + 1],
            )

        def qk_proj(hp):
            # interleave Q and K chunks so the first score pair (needs Q
            # chunk 0 + K chunk 0 only) unblocks as early as possible
            qt = qt_pool.tile([128, s_loc], bf16, tag="qt")
            kt = kt_pool.tile([128, t_len], bf16, tag="kt")
            n_kc = t_len // tch
            for i in range(max(n_qc, n_kc)):
                if i < n_qc:
                    q_chunk(hp, qt, i)
                if i < n_kc:
                    k_chunk(hp, kt, i)
            return qt, kt

        n_vc = c_dim // vch  # V column chunks
        # V tiles are interleaved into the earliest attention blocks (one
        # V tile right before the scores that will consume it) so the exp
        # pipeline starts ~25us earlier instead of waiting on all of V.
        n_vt = n_vc * n_tt  # total V (chunk, tile) pieces
        vt_sched = {}  # (hp, qc, tt) -> (vc, v_tt)
        vi = 0
        for hp in range(n_hp):
            for qc in range(n_qc):
                for tt in range(n_tt):
                    if vi < n_vt:
                        vt_sched[(hp, qc, tt)] = divmod(vi, n_tt)
                        vi += 1
        assert vi >= n_vt, "not enough attention slots to emit V projection"

        for hp in range(n_hp):
            if hp == 0 and n_qc > 1:
                # JIT warmup: emit only the first Q/K chunk up front; the
                # rest go inside the first attention block right before
                # they are needed, so the exp pipeline starts ~15us earlier
                qt = qt_pool.tile([128, s_loc], bf16, tag="qt")
                kt = kt_pool.tile([128, t_len], bf16, tag="kt")
                q_chunk(0, qt, 0)
                k_chunk(0, kt, 0)
            else:
                qt, kt = qk_proj(hp)
            if debug_taps and hp == 0:
                nc.sync.dma_start(out=dbg["qt0"], in_=qt)
                nc.sync.dma_start(out=dbg["kt0"], in_=kt)

            for qc in range(n_qc):
                o_a = o_ps_pool.tile([65, qch], f32, tag="o")
                o_b = o_ps_pool.tile([65, qch], f32, tag="o")
                for tt in range(n_tt):
                    if hp == 0 and qc == 0 and n_qc > 1:
                        if tt in (2, 6, 10) and tt // 4 + 1 < t_len // tch:
                            k_chunk(0, kt, tt // 4 + 1)
                        if tt in (4, 8, 12) and tt // 4 < n_qc:
                            q_chunk(0, qt, tt // 4)
                    if (hp, qc, tt) in vt_sched:
                        vc_i, v_tt = vt_sched[(hp, qc, tt)]
                        v_proj_tile(vc_i, v_tt)
                    if hp == n_hp - 1 and qc > 0 and tt % 4 == 3:
                        # project out the PREVIOUS qc block (its oT columns
                        # are final and its normalization round-trip has had
                        # a full block to complete, so the PE never stalls
                        # on it); spread across the tt loop to keep the
                        # exp pipeline fed
                        out_proj_block((qc - 1) * (qch // 128) + tt // 4)
                    sc_tile = sc_ps.tile([128, 2, qch], f32, tag="sc")
                    # scoresT for head a (contraction rows 0:64) and head b
                    # (rows 64:128) — concurrent via PE row tiling.
                    nc.tensor.matmul(
                        out=sc_tile[:, 0, :],
                        lhsT=kt[0:64, tt * 128 : (tt + 1) * 128],
                        rhs=qt[0:64, qc * qch : (qc + 1) * qch],
                        start=True,
                        stop=True,
                    )
                    nc.tensor.matmul(
                        out=sc_tile[:, 1, :],
                        lhsT=kt[64:128, tt * 128 : (tt + 1) * 128],
                        rhs=qt[64:128, qc * qch : (qc + 1) * qch],
                        start=True,
                        stop=True,
                    )
                    exp_t = exp_pool.tile([128, 2, qch], bf16, tag="exp")
                    nc.scalar.activation(
                        out=exp_t,
                        in_=sc_tile,
                        func=_mybir().ActivationFunctionType.Exp,
                        scale=0.125,
                    )
                    if debug_taps and hp == 0 and qc == 0 and tt == 0:
                        nc.sync.dma_start(out=dbg["exp0"], in_=exp_t)
                    first, last = tt == 0, tt == n_tt - 1
                    # attnV with ones column: row 64 = softmax denominator
                    nc.tensor.matmul(
                        out=o_a,
                        lhsT=v_sb[:, tt, 2 * hp, :],
                        rhs=exp_t[:, 0, :],
                        start=first,
                        stop=last,
                    )
                    nc.tensor.matmul(
                        out=o_b,
                        lhsT=v_sb[:, tt, 2 * hp + 1, :],
                        rhs=exp_t[:, 1, :],
                        start=first,
                        stop=last,
                    )
                # psum evacuation on DVE (ACT is saturated by the exps),
                # then normalize via a DRAM-round-trip partition broadcast
                # of the sums + approx reciprocal (partition-0 based; the op
                # silently no-ops on nonzero partition bases).
                # The very last block is pipelined in column halves so its
                # output projection starts ~5us earlier (this chain + the
                # trailing out-proj are the kernel tail).
                final = hp == n_hp - 1 and qc == n_qc - 1
                halves = 2 if final else 1
                hw_ = qch // halves
                osc_a = osc_pool.tile([65, qch], f32, tag="osc")
                osc_b = osc_pool.tile([65, qch], f32, tag="osc")
                for h in range(halves):
                    cs = slice(h * hw_, (h + 1) * hw_)
                    nc.vector.tensor_copy(out=osc_a[:, cs], in_=o_a[:, cs])
                    nc.vector.tensor_copy(out=osc_b[:, cs], in_=o_b[:, cs])
                    scr = scr_pool.tile([2, qch], f32, tag="scr")
                    nc.sync.dma_start(out=scr[0:1, cs], in_=osc_a[64:65, cs])
                    nc.sync.dma_start(out=scr[1:2, cs], in_=osc_b[64:65, cs])
                    rbc_a = rbc_pool.tile([64, qch], f32, tag="rbc")
                    nc.gpsimd.dma_start(
                        out=rbc_a[:, cs], in_=scr[0:1, cs].to_broadcast([64, hw_])
                    )
                    rbc_b = rbc_pool.tile([64, qch], f32, tag="rbc")
                    nc.gpsimd.dma_start(
                        out=rbc_b[:, cs], in_=scr[1:2, cs].to_broadcast([64, hw_])
                    )
                    nc.vector.reciprocal_approx_fast(
                        out=rbc_a[:, cs], in_=rbc_a[:, cs]
                    )
                    nc.vector.reciprocal_approx_fast(
                        out=rbc_b[:, cs], in_=rbc_b[:, cs]
                    )
                    nc.vector.tensor_mul(
                        oT_all[0:64, hp, qc * qch + h * hw_ : qc * qch + (h + 1) * hw_],
                        osc_a[0:64, cs],
                        rbc_a[:, cs],
                    )
                    nc.vector.tensor_mul(
                        oT_all[64:128, hp, qc * qch + h * hw_ : qc * qch + (h + 1) * hw_],
                        osc_b[0:64, cs],
                        rbc_b[:, cs],
                    )
                    if final:
                        # this half's rows are done for every head: project
                        st0 = qc * (qch // 128) + h * (hw_ // 128)
                        for st in range(st0, st0 + hw_ // 128):
                            out_proj_block(st)

        if debug_taps:
            nc.sync.dma_start(out=dbg["v"], in_=v_sb)
            nc.sync.dma_start(out=dbg["oT"], in_=oT_all)

    nc.compile()
    return nc


def _bf16(a):
    import ml_dtypes

    return np.ascontiguousarray(a).astype(ml_dtypes.bfloat16)


def host_prep_shared(Wq, bq, Wk, bk, Wv, bv, Wo, bo, h0=0, n_hp=4, n_et=8):
    """Pack the weights for heads [h0, h0+2*n_hp) into the kernel's DRAM
    layouts. bo=None packs a zero output bias (exactly one core of each
    partial-sum pair carries the real bias)."""
    e_dim = 128 * n_et
    h1 = h0 + 2 * n_hp

    def pack_pairs(W):
        # [H, E, D] -> [p, hp, et, m] with m = j*64+d, head = h0 + 2*hp+j
        Wr = W[h0:h1].reshape(n_hp, 2, e_dim, D)  # hp, j, e, d
        arr = Wr.transpose(2, 0, 1, 3).reshape(e_dim, n_hp, 128)  # e, hp, m
        arr = arr.reshape(n_et, 128, n_hp, 128).transpose(1, 2, 0, 3)
        return np.ascontiguousarray(arr)  # [p, hp, et, m]

    def bias_cols(b):
        # [H, D] -> [p, hp] with p = j*64+d
        return np.ascontiguousarray(
            b[h0:h1].reshape(n_hp, 2, 64).transpose(1, 2, 0).reshape(128, n_hp)
        ).astype(np.float32)

    c_dim = 128 * n_hp
    wv_cat = Wv[h0:h1].transpose(1, 0, 2).reshape(e_dim, c_dim)  # [e, c]
    wv_arr = wv_cat.reshape(n_et, 128, c_dim).transpose(1, 0, 2)  # [p, et, c]
    wo_arr = (
        Wo[h0 * D : h1 * D].reshape(n_hp, 128, e_dim).transpose(1, 0, 2)
    )  # [p, ct, e]
    bo_arr = np.zeros((1, e_dim), np.float32) if bo is None else bo.reshape(1, e_dim)

    return {
        "wq2": _bf16(pack_pairs(Wq)),
        "wk2": _bf16(pack_pairs(Wk)),
        "wv": _bf16(np.ascontiguousarray(wv_arr)),
        "wo": _bf16(np.ascontiguousarray(wo_arr)),
        "bqc": bias_cols(bq),
        "bkc": bias_cols(bk),
        "bvc": _bf16(bv[h0:h1].reshape(1, c_dim)),
        "bo_in": np.ascontiguousarray(bo_arr).astype(np.float32),
    }


def host_prep_xt(mat, n_et=8):
    """[rows, E] -> [p, et, rows] transposed tiled layout, bf16."""
    rows, e_dim = mat.shape
    assert e_dim == 128 * n_et
    arr = mat.T.reshape(n_et, 128, rows).transpose(1, 0, 2)
    return _bf16(arr)


def kernel(x, y, Wq, bq, Wk, bk, Wv, bv, Wo, bo):
    import os
    import sys

    if "/opt/trn_rl_repo" not in sys.path:
        sys.path.insert(0, "/opt/trn_rl_repo")
    from concourse import bass_utils

    x = np.asarray(x, dtype=np.float32)
    y = np.asarray(y, dtype=np.float32)

    if "prog" not in _compiled:
        _compiled["prog"] = build_program()
    nc = _compiled["prog"]

    args = [
        np.asarray(Wq, np.float32),
        np.asarray(bq, np.float32),
        np.asarray(Wk, np.float32),
        np.asarray(bk, np.float32),
        np.asarray(Wv, np.float32),
        np.asarray(bv, np.float32),
        np.asarray(Wo, np.float32),
    ]
    bo = np.asarray(bo, np.float32)
    # core c: batch c//2, heads (c%2)*8 : (c%2)*8+8; the pair's partial
    # outputs are summed on the host (TP unshard over combo_linear input)
    shared_h = [
        host_prep_shared(*args, bo if hh == 0 else None, h0=8 * hh) for hh in (0, 1)
    ]
    xT_b = [host_prep_xt(x[b]) for b in range(B)]
    yT_b = [host_prep_xt(y[b]) for b in range(B)]
    in_maps = []
    for c in range(N_CORES):
        b, hh = c // 2, c % 2
        m = dict(shared_h[hh])
        m["xT"] = xT_b[b]
        m["yT"] = yT_b[b]
        in_maps.append(m)

    trace = os.environ.get("TRN_ATTN_TRACE", "0") == "1"
    res = bass_utils.run_bass_kernel_spmd(
        nc, in_maps, core_ids=list(range(N_CORES)), trace=trace
    )
    _compiled["last_results"] = res
    out = np.empty((B, S, E), dtype=np.float32)
    for b in range(B):
        np.add(
            res.results[2 * b]["out"], res.results[2 * b + 1]["out"], out=out[b]
        )
    return out

